# revision 7
# baseline (speedup 1.0000x reference)
"""Engram block (hash-embedding gather + gated value + dilated causal depthwise
conv) as a Bass/Tile SPMD kernel on 8 Trainium2 NeuronCores.

Sharding: sequence (L) split 8 ways; each core recomputes a 12-position halo
for the causal conv. Embedding tables are replicated (the gather reads only
needed rows). Weights host-transposed/cast to bf16.

v2 pipeline (per 128-token m-tile):
  1. ONE fused indirect-DMA gather of all 12 head embeddings (GPSIMD SWDGE)
     -> XBAR DMA transpose to embT [dh, h, m] (no PE involvement).
  2. K matmul (12 e-tiles x 2048 cols, 4 PSUM banks), stats on PSUM
     (ACT square-accum + DVE dot with hidden), gate tail on [128,1].
  3. V matmul (other 4 PSUM banks) overlaps the gate tail; DVE applies the
     per-token gate, XBAR DMA transpose back to vbig [d, m].
  4. Dilated causal conv = 4 free-dim-shifted DVE FMAs over vbig, emitted in
     wide multi-tile chunks; one 3D-AP output DMA per chunk.
PE does matmuls only; transposes ride the DMA engines.
"""
import sys

sys.path.insert(0, "/opt/trn_rl_repo")

import numpy as np
import ml_dtypes

import concourse.bass as bass
import concourse.tile as tile
from concourse import mybir
from concourse.bass_utils import run_bass_kernel_spmd

# problem shapes (hardcoded per spec)
L, B, D = 4096, 2, 2048
H, Dh = 12, 128
E = H * Dh  # 1536
N = 100000
K, DIL = 4, 4
EPS = 1e-6

NCORES = 8
LC = L // NCORES          # 512 l-positions per core
HALO = (K - 1) * DIL      # 12
LE = LC + HALO            # 524
M = LE * B                # 1048 valid tokens (l-major, b inner)
MP = 1152                 # padded to 9*128
MT = MP // 128            # 9 m-tiles
DT = D // 128             # 16 d-tiles
ET = E // 128             # 12 e-tiles
MOUT = LC * B             # 1024 output tokens per core
OFF = HALO * B            # 24 = first valid output token
D2 = 2 * D                # concat k|v output cols

# conv chunk emitted in iteration t covers out cols [(t-1)*128-24, t*128-24)
# (clamped); it reads v cols < t*128, i.e. tiles transposed >= 1 iter earlier

BF16 = mybir.dt.bfloat16
F32 = mybir.dt.float32
I32 = mybir.dt.int32

# scal columns per d-tile
SC_W0, SC_W1, SC_W2, SC_W3P, SC_CB = range(5)
NSC = 5


def _split_multi_waits(nc):
    """This walrus build accepts only one sync-wait per instruction; hoist
    extra waits onto injected NOPs on the same engine (order-preserving)."""
    for f in nc.m.functions:
        for bb in f.blocks:
            new_insts = []
            for inst in bb.instructions:
                si = inst.sync_info
                if si is not None and si.on_wait and len(si.on_wait) > 1:
                    for w in si.on_wait[:-1]:
                        nop = mybir.InstNoOp(
                            name=nc.get_next_instruction_name(), ins=[], outs=[]
                        )
                        nop.engine = inst.engine
                        nop.sync_info = mybir.SyncInfo(on_wait=[w], on_update=[])
                        new_insts.append(nop)
                    si.on_wait = [si.on_wait[-1]]
                new_insts.append(inst)
            bb.instructions = new_insts


def build_program():
    nc = bass.Bass("TRN2", target_bir_lowering=False, debug=False)

    tabs = nc.declare_dram_parameter("tabs", [H * N, Dh], BF16, isOutput=False)
    ids = nc.declare_dram_parameter("ids", [128, MT * H], I32, isOutput=False)
    hid = nc.declare_dram_parameter("hid", [MP, D], BF16, isOutput=False)
    wkv = nc.declare_dram_parameter("wkv", [E, D2], BF16, isOutput=False)
    scal = nc.declare_dram_parameter("scal", [128, DT * NSC], F32, isOutput=False)
    outT = nc.declare_dram_parameter("outT", [D, MOUT], F32, isOutput=True)

    AR = mybir.ActivationFunctionType
    ALU = mybir.AluOpType

    with tile.TileContext(nc) as tc:
        with (
            tc.tile_pool(name="persist", bufs=1) as pp,
            tc.tile_pool(name="work", bufs=3) as wp,
            tc.tile_pool(name="stat", bufs=2) as sp,
            tc.tile_pool(name="psum", bufs=1, space="PSUM") as psp,
        ):
            # ---- constants / small inputs ----
            eps_sb = pp.tile([128, 1], F32, tag="eps")
            nc.vector.memset(eps_sb[:], EPS)

            ids_sb = pp.tile([128, MT * H], I32, tag="ids")
            nc.sync.dma_start(ids_sb[:], ids.ap())
            scal_sb = pp.tile([128, DT * NSC], F32, tag="scal")
            nc.sync.dma_start(scal_sb[:], scal.ap())

            def sc(dt_, c):
                return scal_sb[:, dt_ * NSC + c : dt_ * NSC + c + 1]

            # ---- weights (resident, concat k|v along cols); stream the K
            #      halves first so K matmuls of early tiles aren't gated on
            #      the V halves ----
            wkv_sb = []
            for e in range(ET):
                w = pp.tile([128, D2], BF16, tag=f"wkv{e}", name=f"wkv{e}")
                nc.scalar.dma_start(w[:, 0:D], wkv[e * 128 : (e + 1) * 128, 0:D])
                wkv_sb.append(w)
            for e in range(ET):
                nc.scalar.dma_start(
                    wkv_sb[e][:, D:D2], wkv[e * 128 : (e + 1) * 128, D:D2]
                )

            # ---- gathers + XBAR transposes, all ahead of compute ----
            bc_reg = nc.gpsimd.to_reg(H * N - 1)
            embT = []  # per m-tile [128(dh), H, 128(m)]
            for t in range(MT):
                er = wp.tile([128, E], BF16, tag="emb_raw", bufs=3,
                             name=f"emb_raw{t}")
                if t in (0, MT - 1):
                    nc.gpsimd.memset(er[:], 0)
                nc.gpsimd.indirect_dma_start(
                    out=er[:],
                    out_offset=None,
                    in_=tabs[:],
                    in_offset=bass.IndirectOffsetOnAxis(
                        ap=ids_sb[:, t * H : (t + 1) * H], axis=0
                    ),
                    bounds_check=bc_reg,
                    oob_is_err=False,
                )
                et = pp.tile([128, H, 128], BF16, tag="embT", bufs=4,
                             name=f"embT{t}")
                nc.sync.dma_start(et[:, :, :], er[:], transpose=True)
                embT.append(et)

            vbig = pp.tile([128, DT, MP], BF16, tag="vbig")
            g_stats = pp.tile([128, MT], F32, tag="g_stats")

            psK = psp.tile([128, D], F32, tag="psK", space="PSUM")
            psV = psp.tile([128, D], F32, tag="psV", space="PSUM")

            def conv_chunk(c0, cw):
                """Emit conv + output DMA for out cols [c0, c0+cw)."""
                ot = wp.tile([128, DT, 128], F32, tag="ot", bufs=2)
                for dt_ in range(DT):
                    vs = vbig[:, dt_, :]
                    a1 = wp.tile([128, cw], BF16, tag="a1", bufs=2)
                    nc.vector.tensor_scalar(
                        out=a1[:], in0=vs[:, c0 : c0 + cw],
                        scalar1=sc(dt_, SC_W0), scalar2=sc(dt_, SC_CB),
                        op0=ALU.mult, op1=ALU.add,
                    )
                    a2 = wp.tile([128, cw], BF16, tag="a2", bufs=2)
                    nc.vector.scalar_tensor_tensor(
                        out=a2[:], in0=vs[:, c0 + 8 : c0 + 8 + cw],
                        scalar=sc(dt_, SC_W1), in1=a1[:],
                        op0=ALU.mult, op1=ALU.add,
                    )
                    a3 = wp.tile([128, cw], BF16, tag="a3", bufs=2)
                    nc.vector.scalar_tensor_tensor(
                        out=a3[:], in0=vs[:, c0 + 16 : c0 + 16 + cw],
                        scalar=sc(dt_, SC_W2), in1=a2[:],
                        op0=ALU.mult, op1=ALU.add,
                    )
                    nc.vector.scalar_tensor_tensor(
                        out=ot[:, dt_, 0:cw], in0=vs[:, c0 + OFF : c0 + OFF + cw],
                        scalar=sc(dt_, SC_W3P), in1=a3[:],
                        op0=ALU.mult, op1=ALU.add,
                    )
                nc.sync.dma_start(
                    outT.ap()[:, c0 : c0 + cw].rearrange(
                        "(dt p) x -> p dt x", p=128
                    ),
                    ot[:, :, 0:cw],
                )

            # ---- main per-m-tile pipeline ----
            for t in range(MT):
                # hidden rows for this m-tile + h^2 accum
                h_md = wp.tile([128, D], BF16, tag="h_md", bufs=2)
                nc.sync.dma_start(h_md[:], hid.ap()[t * 128 : (t + 1) * 128, :])

                # K matmul: 12 e-tiles x 4 psum banks
                for e in range(ET):
                    for b in range(4):
                        nc.tensor.matmul(
                            out=psK[:, b * 512 : (b + 1) * 512],
                            lhsT=embT[t][:, e, :],
                            rhs=wkv_sb[e][:, b * 512 : (b + 1) * 512],
                            start=(e == 0), stop=(e == ET - 1),
                        )

                sh = sp.tile([128, 1], F32, tag="sh")
                hsj = wp.tile([128, D], BF16, tag="junk", bufs=2, name=f"hsj{t}")
                nc.scalar.activation(
                    out=hsj[:], in_=h_md[:], func=AR.Square, accum_out=sh[:]
                )

                # k stats straight off PSUM
                sk = sp.tile([128, 1], F32, tag="sk")
                ksj = wp.tile([128, D], BF16, tag="junk", bufs=2, name=f"ksj{t}")
                nc.scalar.activation(
                    out=ksj[:], in_=psK[:], func=AR.Square, accum_out=sk[:]
                )
                pk = sp.tile([128, 1], F32, tag="pk")
                khj = wp.tile([128, D], BF16, tag="junk", bufs=2, name=f"khj{t}")
                nc.vector.scalar_tensor_tensor(
                    out=khj[:], in0=psK[:], scalar=1.0, in1=h_md[:],
                    op0=ALU.mult, op1=ALU.mult, accum_out=pk[:],
                )

                # V matmul (other 4 banks) — overlaps gate tail below
                for e in range(ET):
                    for b in range(4):
                        nc.tensor.matmul(
                            out=psV[:, b * 512 : (b + 1) * 512],
                            lhsT=embT[t][:, e, :],
                            rhs=wkv_sb[e][:, D + b * 512 : D + (b + 1) * 512],
                            start=(e == 0), stop=(e == ET - 1),
                        )

                # gate tail on [128,1]
                s1 = sp.tile([128, 1], F32, tag="s1")
                nc.scalar.activation(
                    out=s1[:], in_=sk[:], func=AR.Identity,
                    bias=eps_sb[:, 0:1], scale=1.0 / D,
                )
                s2 = sp.tile([128, 1], F32, tag="s2")
                nc.scalar.activation(
                    out=s2[:], in_=sh[:], func=AR.Identity,
                    bias=eps_sb[:, 0:1], scale=1.0 / D,
                )
                tt = sp.tile([128, 1], F32, tag="tt")
                nc.vector.tensor_mul(tt[:], s1[:], s2[:])
                rr = sp.tile([128, 1], F32, tag="rr")
                nc.vector.reciprocal(rr[:], tt[:])
                rq = sp.tile([128, 1], F32, tag="rq")
                nc.scalar.activation(out=rq[:], in_=rr[:], func=AR.Sqrt)
                uu = sp.tile([128, 1], F32, tag="uu")
                nc.vector.scalar_tensor_tensor(
                    out=uu[:], in0=pk[:], scalar=float(1.0 / np.sqrt(D)),
                    in1=rq[:], op0=ALU.mult, op1=ALU.mult,
                )
                ab = sp.tile([128, 1], F32, tag="ab")
                nc.scalar.activation(out=ab[:], in_=uu[:], func=AR.Abs)
                mx = sp.tile([128, 1], F32, tag="mx")
                nc.vector.tensor_scalar_max(out=mx[:], in0=ab[:], scalar1=1e-6)
                r2 = sp.tile([128, 1], F32, tag="r2")
                nc.vector.reciprocal(r2[:], mx[:])
                q2 = sp.tile([128, 1], F32, tag="q2")
                nc.scalar.activation(out=q2[:], in_=r2[:], func=AR.Sqrt)
                st = sp.tile([128, 1], F32, tag="st")
                nc.vector.tensor_mul(st[:], uu[:], q2[:])
                nc.scalar.activation(
                    out=g_stats[:, t : t + 1], in_=st[:], func=AR.Sigmoid
                )

                # gated value -> v_md [m, d] bf16 -> XBAR transpose to vbig
                v_md = wp.tile([128, D], BF16, tag="v_md", bufs=2)
                nc.vector.tensor_scalar_mul(
                    out=v_md[:], in0=psV[:], scalar1=g_stats[:, t : t + 1]
                )
                nc.sync.dma_start(
                    vbig[:, :, t * 128 : (t + 1) * 128], v_md[:], transpose=True
                )

                if t >= 1:
                    c0 = max(0, (t - 1) * 128 - 24)
                    conv_chunk(c0, t * 128 - 24 - c0)

            conv_chunk(1000, 24)

    _split_multi_waits(nc)
    return nc


_CACHE = {}


def _get_program():
    if "nc" not in _CACHE:
        _CACHE["nc"] = build_program()
    return _CACHE["nc"]


def host_prep(hidden_states, hash_input_ids, emb_tables, key_w, key_b,
              norm1_w, norm2_w, value_w, value_b, conv_w, conv_b):
    """Shard + lay out inputs for the 8 cores. Returns in_maps list."""
    bf = ml_dtypes.bfloat16
    w12 = norm1_w.astype(np.float64) * norm2_w.astype(np.float64)
    assert np.allclose(w12, 1.0, atol=1e-5), (
        "fast path assumes norm1_w*norm2_w == 1 (problem spec: fill=ones)"
    )
    assert not key_b.any() and not value_b.any(), (
        "fast path assumes zero key/value biases (problem spec: fill=zeros)"
    )

    tabs_np = np.ascontiguousarray(emb_tables.reshape(H * N, Dh)).astype(bf)
    wkv_np = np.empty((E, D2), bf)
    wkv_np[:, :D] = key_w.T.astype(bf)
    wkv_np[:, D:] = value_w.T.astype(bf)
    scal_d = np.empty((D, NSC), np.float32)
    scal_d[:, SC_W0] = conv_w[:, 0]
    scal_d[:, SC_W1] = conv_w[:, 1]
    scal_d[:, SC_W2] = conv_w[:, 2]
    scal_d[:, SC_W3P] = conv_w[:, 3] + 1.0
    scal_d[:, SC_CB] = conv_b
    scal_np = np.ascontiguousarray(
        scal_d.reshape(DT, 128, NSC).transpose(1, 0, 2).reshape(128, DT * NSC)
    )

    head_off = (np.arange(H, dtype=np.int64) * N)[None, :]
    OOB = np.int32(H * N)

    in_maps = []
    for c in range(NCORES):
        l0 = c * LC
        lo = l0 - HALO
        lo_clip = max(lo, 0)
        nvalid = (l0 + LC) - lo_clip
        r0 = (lo_clip - lo) * B
        ids_c = np.full((MP, H), OOB, np.int32)
        seg = hash_input_ids[lo_clip : l0 + LC].reshape(nvalid * B, H)
        ids_c[r0 : r0 + nvalid * B] = (seg.astype(np.int64) + head_off).astype(
            np.int32
        )
        hid_c = np.zeros((MP, D), bf)
        hseg = hidden_states[lo_clip : l0 + LC].reshape(nvalid * B, D)
        hid_c[r0 : r0 + nvalid * B] = hseg.astype(bf)
        ids_r = np.ascontiguousarray(
            ids_c.reshape(MT, 128, H).transpose(1, 0, 2).reshape(128, MT * H)
        )
        in_maps.append(
            {
                "tabs": tabs_np,
                "ids": ids_r,
                "hid": hid_c,
                "wkv": wkv_np,
                "scal": scal_np,
            }
        )
    return in_maps


def unshard_output(results):
    """results: list of per-core dicts with 'outT' [D, MOUT] -> [L, B, D]."""
    out = np.empty((L, B, D), np.float32)
    for c in range(NCORES):
        o = results[c]["outT"]
        out[c * LC : (c + 1) * LC] = o.reshape(D, LC, B).transpose(1, 2, 0)
    return out


def kernel(hidden_states, hash_input_ids, emb_tables, key_w, key_b,
           norm1_w, norm2_w, value_w, value_b, conv_w, conv_b):
    args = [hidden_states, hash_input_ids, emb_tables, key_w, key_b,
            norm1_w, norm2_w, value_w, value_b, conv_w, conv_b]
    args = [np.asarray(a) for a in args]
    in_maps = host_prep(*args)
    nc = _get_program()
    res = run_bass_kernel_spmd(nc, in_maps, list(range(NCORES)))
    return unshard_output(res.results)


# revision 52
# speedup vs baseline: 1.0873x; 1.0873x over previous
"""Engram block (hash-embedding gather + gated value + dilated causal depthwise
conv) as a Bass/Tile SPMD kernel on 8 Trainium2 NeuronCores.

Sharding: sequence (L) split 8 ways; each core recomputes a 12-position halo
for the causal conv. Embedding tables are replicated (the gather reads only
needed rows). Weights host-transposed/cast to bf16.

Per-core pipeline (per 128-token m-tile, so PE overlaps the gather):
  1. indirect-DMA gather of 12 head embeddings -> PE transpose -> embT [e, m]
  2. k|v projections as ONE matmul family: stationary = embT block (one
     LDWEIGHTS per 1024 streamed cols), moving = concat [Wk^T | Wv^T] cols;
     PSUM out is [m_tile, d_cols], so RMS/gate stats are free-dim reductions
     (ACT square-accumulate, DVE tensor_tensor_reduce) and the gate applies
     as a per-partition scalar.
  3. gated value transposed back (PE) to [d, m] for the dilated conv, which
     is 4 free-dim-shifted fused multiply-adds on DVE; fp32 result DMA'd out
     as [D, m_out] (host re-transposes when unsharding).
"""
import sys

sys.path.insert(0, "/opt/trn_rl_repo")

import numpy as np
import ml_dtypes

import concourse.bass as bass
import concourse.tile as tile
from concourse import mybir
from concourse.masks import make_identity
from concourse.bass_utils import run_bass_kernel_spmd

# problem shapes (hardcoded per spec)
L, B, D = 4096, 2, 2048
H, Dh = 12, 128
E = H * Dh  # 1536
N = 100000
K, DIL = 4, 4
EPS = 1e-6

NCORES = 8
LC = L // NCORES          # 512 l-positions per core
HALO = (K - 1) * DIL      # 12
LE = LC + HALO            # 524
M = LE * B                # 1048 valid tokens (l-major, b inner)
MP = 1152                 # padded to 9*128
MT = MP // 128            # 9 m-tiles
DT = D // 128             # 16 d-tiles
ET = E // 128             # 12 e-tiles
MOUT = LC * B             # 1024 output tokens per core
OFF = HALO * B            # 24 = first valid output token
D2 = 2 * D                # concat k|v output cols
GRP = 1024                # matmul column group (2 PSUM banks)
NGRP = D2 // GRP          # 4
# conv ranges (out-col start, width); range r ready after m-tile LAST_MT[r]
CONV_R = [(0, 488), (488, 232), (720, 256), (976, 48)]

BF16 = mybir.dt.bfloat16
F32 = mybir.dt.float32
I32 = mybir.dt.int32

# scal columns per d-tile
SC_W0, SC_W1, SC_W2, SC_W3P, SC_CB = range(5)
NSC = 5


def _split_multi_waits(nc):
    """This walrus build accepts only one sync-wait per instruction; hoist
    extra waits onto injected NOPs on the same engine (order-preserving)."""
    for f in nc.m.functions:
        for bb in f.blocks:
            new_insts = []
            for inst in bb.instructions:
                si = inst.sync_info
                if si is not None and si.on_wait and len(si.on_wait) > 1:
                    for w in si.on_wait[:-1]:
                        nop = mybir.InstNoOp(
                            name=nc.get_next_instruction_name(), ins=[], outs=[]
                        )
                        nop.engine = inst.engine
                        nop.sync_info = mybir.SyncInfo(on_wait=[w], on_update=[])
                        new_insts.append(nop)
                    si.on_wait = [si.on_wait[-1]]
                new_insts.append(inst)
            bb.instructions = new_insts


def build_program():
    nc = bass.Bass("TRN2", target_bir_lowering=False, debug=False)

    tabs = nc.declare_dram_parameter("tabs", [H * N, Dh], BF16, isOutput=False)
    ids = nc.declare_dram_parameter("ids", [128, MT * H], I32, isOutput=False)
    hid = nc.declare_dram_parameter("hid", [MP, D], BF16, isOutput=False)
    wkv = nc.declare_dram_parameter("wkv", [E, D2], BF16, isOutput=False)
    scal = nc.declare_dram_parameter("scal", [128, DT * NSC], F32, isOutput=False)
    outT = nc.declare_dram_parameter("outT", [D, MOUT], BF16, isOutput=True)

    AR = mybir.ActivationFunctionType
    ALU = mybir.AluOpType

    with tile.TileContext(nc) as tc:
        with (
            tc.tile_pool(name="persist", bufs=1) as pp,
            tc.tile_pool(name="work", bufs=3) as wp,
            tc.tile_pool(name="stat", bufs=2) as sp,
            tc.tile_pool(name="psum", bufs=2, space="PSUM") as psp,
        ):
            # ---- constants / small inputs (ids on the idle sync ring so
            #      gathers are not queued behind the 12MB weight DMAs) ----
            eps_sb = pp.tile([128, 1], F32, tag="eps")
            nc.vector.memset(eps_sb[:], EPS)

            ids_sb = pp.tile([128, MT * H], I32, tag="ids")
            nc.sync.dma_start(ids_sb[:], ids.ap())
            scal_sb = pp.tile([128, DT * NSC], F32, tag="scal")
            nc.sync.dma_start(scal_sb[:], scal.ap())

            def sc(dt_, c):
                return scal_sb[:, dt_ * NSC + c : dt_ * NSC + c + 1]

            # ---- weights (resident, concat k|v along cols) ----
            wkv_sb = []
            for e in range(ET):
                w = pp.tile([128, D2], BF16, tag=f"wkv{e}", name=f"wkv{e}")
                wkv_sb.append(w)
            for e in range(4):
                nc.scalar.dma_start(
                    wkv_sb[e][:, 0:D], wkv[e * 128 : (e + 1) * 128, 0:D]
                )
            with tc.tile_wait_until(0.010):
                for e in range(4, ET):
                    nc.scalar.dma_start(
                        wkv_sb[e][:, 0:D], wkv[e * 128 : (e + 1) * 128, 0:D]
                    )
            with tc.tile_wait_until(0.012):
                for e in range(ET):
                    nc.scalar.dma_start(
                        wkv_sb[e][:, D:D2], wkv[e * 128 : (e + 1) * 128, D:D2]
                    )

            # ---- gather all m-tiles up front (program order sets priority;
            #      Q7/SDMA stream ahead of PE consumption) ----
            bc_reg = nc.gpsimd.to_reg(H * N - 1)
            emb_raws = []
            for t in range(MT):
                er = wp.tile(
                    [128, H * Dh], BF16, tag="emb_raw", bufs=3,
                    name=f"emb_raw{t}",
                )
                if t in (0, MT - 1):
                    nc.gpsimd.memset(er[:], 0)
                for h in range(H):
                    nc.gpsimd.indirect_dma_start(
                        out=er[:, h * Dh : (h + 1) * Dh],
                        out_offset=None,
                        in_=tabs[:],
                        in_offset=bass.IndirectOffsetOnAxis(
                            ap=ids_sb[:, t * H + h : t * H + h + 1], axis=0
                        ),
                        bounds_check=bc_reg,
                        oob_is_err=False,
                    )
                emb_raws.append(er)

            ident = pp.tile([128, 128], BF16, tag="ident")
            make_identity(nc, ident[:])
            embT = [
                pp.tile([128, MP], BF16, tag=f"embT{h}", name=f"embT{h}")
                for h in range(H)
            ]
            v_sb = [
                pp.tile([128, MP], BF16, tag=f"v_sb{dt_}", name=f"v_sb{dt_}")
                for dt_ in range(DT)
            ]
            g_stats = pp.tile([128, MT], F32, tag="g_stats")  # gate G per m-tile

            def conv_range(r):
                """Emit conv + output DMA for out-col range r (all d-tiles)."""
                c0, cw = CONV_R[r]
                for dt_ in range(DT):
                    vs = v_sb[dt_]
                    a1 = wp.tile([128, cw], BF16, tag="a1", bufs=2)
                    nc.vector.tensor_scalar(
                        out=a1[:], in0=vs[:, c0 : c0 + cw],
                        scalar1=sc(dt_, SC_W0), scalar2=sc(dt_, SC_CB),
                        op0=ALU.mult, op1=ALU.add,
                    )
                    a2 = wp.tile([128, cw], BF16, tag="a2", bufs=2)
                    nc.vector.scalar_tensor_tensor(
                        out=a2[:], in0=vs[:, c0 + 8 : c0 + 8 + cw],
                        scalar=sc(dt_, SC_W1), in1=a1[:],
                        op0=ALU.mult, op1=ALU.add,
                    )
                    a3 = wp.tile([128, cw], BF16, tag="a3", bufs=2)
                    nc.vector.scalar_tensor_tensor(
                        out=a3[:], in0=vs[:, c0 + 16 : c0 + 16 + cw],
                        scalar=sc(dt_, SC_W2), in1=a2[:],
                        op0=ALU.mult, op1=ALU.add,
                    )
                    ot = wp.tile([128, cw], BF16, tag="ot", bufs=2)
                    nc.vector.scalar_tensor_tensor(
                        out=ot[:], in0=vs[:, c0 + OFF : c0 + OFF + cw],
                        scalar=sc(dt_, SC_W3P), in1=a3[:],
                        op0=ALU.mult, op1=ALU.add,
                    )
                    nc.sync.dma_start(
                        outT[dt_ * 128 : (dt_ + 1) * 128, c0 : c0 + cw], ot[:]
                    )

            # ---- main per-m-tile pipeline ----
            for t in range(MT):
                er = emb_raws[t]
                # transpose 12 head blocks -> embT
                for h in range(H):
                    pt = psp.tile([128, 128], BF16, tag="tpose", space="PSUM")
                    nc.tensor.transpose(
                        out=pt[:], in_=er[:, h * Dh : (h + 1) * Dh],
                        identity=ident[:],
                    )
                    nc.scalar.copy(
                        out=embT[h][:, t * 128 : (t + 1) * 128], in_=pt[:]
                    )

                # hidden rows for this m-tile (natural layout) + h^2 accum
                h_md = wp.tile([128, D], BF16, tag="h_md", bufs=2)
                nc.sync.dma_start(h_md[:], hid.ap()[t * 128 : (t + 1) * 128, :])
                sh = sp.tile([128, 1], F32, tag="sh")
                hsj = wp.tile([128, D], BF16, tag="junk", bufs=2, name="hsj")
                nc.scalar.activation(
                    out=hsj[:], in_=h_md[:], func=AR.Square, accum_out=sh[:]
                )

                # k|v matmuls in 4 col-groups of 1024 (2 PSUM banks each)
                sk_p = sp.tile([128, NGRP // 2], F32, tag="sk_p")
                pk_c = [sp.tile([128, 1], F32, tag=f"pk{i}", name=f"pk{i}_{t}")
                        for i in range(2)]
                vglo = []
                v_md = wp.tile([128, D], BF16, tag="v_md", bufs=2)
                for g in range(NGRP):
                    mm_ps = psp.tile([128, GRP], F32, tag="mm_ps", bufs=3, space="PSUM")
                    for e in range(ET):
                        for b in range(GRP // 512):
                            nc.tensor.matmul(
                                out=mm_ps[:, b * 512 : (b + 1) * 512],
                                lhsT=embT[e][:, t * 128 : (t + 1) * 128],
                                rhs=wkv_sb[e][:, g * GRP + b * 512 :
                                              g * GRP + (b + 1) * 512],
                                start=(e == 0), stop=(e == ET - 1),
                            )
                    if g < 2:
                        # k stats: sum k^2 (ACT), sum k*h (DVE ttr chain)
                        ksj = wp.tile([128, GRP], BF16, tag="junk", bufs=2, name="ksj")
                        nc.scalar.activation(
                            out=ksj[:], in_=mm_ps[:], func=AR.Square,
                            accum_out=sk_p[:, g : g + 1],
                        )
                        khj = wp.tile([128, GRP], BF16, tag="junk", bufs=2, name="khj")
                        nc.vector.scalar_tensor_tensor(
                            out=khj[:], in0=mm_ps[:], scalar=1.0,
                            in1=h_md[:, g * GRP : (g + 1) * GRP],
                            op0=ALU.mult, op1=ALU.mult,
                            accum_out=pk_c[g][:],
                        )
                    else:
                        vglo.append(mm_ps)

                # gate tail for this m-tile on [128,1]
                s1 = sp.tile([128, 1], F32, tag="s1")
                nc.scalar.activation(
                    out=s1[:], in_=sk_p[:, 0:1], func=AR.Identity,
                    bias=eps_sb[:, 0:1], scale=1.0 / D,
                )
                # add second k^2 part: s1 += sk_p[:,1]/D  (fold via stt)
                s1b = sp.tile([128, 1], F32, tag="s1b")
                nc.vector.scalar_tensor_tensor(
                    out=s1b[:], in0=sk_p[:, 1:2], scalar=1.0 / D, in1=s1[:],
                    op0=ALU.mult, op1=ALU.add,
                )
                s2 = sp.tile([128, 1], F32, tag="s2")
                nc.scalar.activation(
                    out=s2[:], in_=sh[:], func=AR.Identity,
                    bias=eps_sb[:, 0:1], scale=1.0 / D,
                )
                tt = sp.tile([128, 1], F32, tag="tt")
                nc.vector.tensor_mul(tt[:], s1b[:], s2[:])
                rr = sp.tile([128, 1], F32, tag="rr")
                nc.vector.reciprocal(rr[:], tt[:])
                rq = sp.tile([128, 1], F32, tag="rq")
                nc.scalar.activation(out=rq[:], in_=rr[:], func=AR.Sqrt)
                pks = sp.tile([128, 1], F32, tag="pks")
                nc.vector.tensor_add(pks[:], pk_c[0][:], pk_c[1][:])
                uu = sp.tile([128, 1], F32, tag="uu")
                nc.vector.scalar_tensor_tensor(
                    out=uu[:], in0=pks[:], scalar=float(1.0 / np.sqrt(D)),
                    in1=rq[:], op0=ALU.mult, op1=ALU.mult,
                )
                ab = sp.tile([128, 1], F32, tag="ab")
                nc.scalar.activation(out=ab[:], in_=uu[:], func=AR.Abs)
                mx = sp.tile([128, 1], F32, tag="mx")
                nc.vector.tensor_scalar_max(out=mx[:], in0=ab[:], scalar1=1e-6)
                r2 = sp.tile([128, 1], F32, tag="r2")
                nc.vector.reciprocal(r2[:], mx[:])
                q2 = sp.tile([128, 1], F32, tag="q2")
                nc.scalar.activation(out=q2[:], in_=r2[:], func=AR.Sqrt)
                st = sp.tile([128, 1], F32, tag="st")
                nc.vector.tensor_mul(st[:], uu[:], q2[:])
                nc.scalar.activation(
                    out=g_stats[:, t : t + 1], in_=st[:], func=AR.Sigmoid
                )

                # gated value -> v_md [m, d] bf16, transposed per group so
                # the PE transposes interleave with later matmul groups
                for gi, vp in enumerate(vglo):
                    nc.vector.tensor_scalar_mul(
                        out=v_md[:, gi * GRP : (gi + 1) * GRP], in0=vp[:],
                        scalar1=g_stats[:, t : t + 1],
                    )
                    for dt_ in range(gi * 8, (gi + 1) * 8):
                        pt = psp.tile([128, 128], BF16, tag="tpose", space="PSUM")
                        nc.tensor.transpose(
                            out=pt[:], in_=v_md[:, dt_ * 128 : (dt_ + 1) * 128],
                            identity=ident[:],
                        )
                        nc.scalar.copy(
                            out=v_sb[dt_][:, t * 128 : (t + 1) * 128], in_=pt[:]
                        )

                if t == 4:
                    conv_range(0)
                if t == 6:
                    conv_range(1)
                if t == 8:
                    conv_range(2)
            conv_range(3)

    _split_multi_waits(nc)
    return nc


_CACHE = {}


def _get_program():
    if "nc" not in _CACHE:
        _CACHE["nc"] = build_program()
    return _CACHE["nc"]


def host_prep(hidden_states, hash_input_ids, emb_tables, key_w, key_b,
              norm1_w, norm2_w, value_w, value_b, conv_w, conv_b):
    """Shard + lay out inputs for the 8 cores. Returns in_maps list."""
    bf = ml_dtypes.bfloat16
    w12 = norm1_w.astype(np.float64) * norm2_w.astype(np.float64)
    assert np.allclose(w12, 1.0, atol=1e-5), (
        "fast path assumes norm1_w*norm2_w == 1 (problem spec: fill=ones)"
    )
    assert not key_b.any() and not value_b.any(), (
        "fast path assumes zero key/value biases (problem spec: fill=zeros)"
    )

    tabs_np = np.ascontiguousarray(emb_tables.reshape(H * N, Dh)).astype(bf)
    wkv_np = np.empty((E, D2), bf)
    wkv_np[:, :D] = key_w.T.astype(bf)
    wkv_np[:, D:] = value_w.T.astype(bf)
    scal_d = np.empty((D, NSC), np.float32)
    scal_d[:, SC_W0] = conv_w[:, 0]
    scal_d[:, SC_W1] = conv_w[:, 1]
    scal_d[:, SC_W2] = conv_w[:, 2]
    scal_d[:, SC_W3P] = conv_w[:, 3] + 1.0
    scal_d[:, SC_CB] = conv_b
    scal_np = np.ascontiguousarray(
        scal_d.reshape(DT, 128, NSC).transpose(1, 0, 2).reshape(128, DT * NSC)
    )

    head_off = (np.arange(H, dtype=np.int64) * N)[None, :]
    OOB = np.int32(H * N)

    in_maps = []
    for c in range(NCORES):
        l0 = c * LC
        lo = l0 - HALO
        lo_clip = max(lo, 0)
        nvalid = (l0 + LC) - lo_clip
        r0 = (lo_clip - lo) * B
        ids_c = np.full((MP, H), OOB, np.int32)
        seg = hash_input_ids[lo_clip : l0 + LC].reshape(nvalid * B, H)
        ids_c[r0 : r0 + nvalid * B] = (seg.astype(np.int64) + head_off).astype(
            np.int32
        )
        hid_c = np.zeros((MP, D), bf)
        hseg = hidden_states[lo_clip : l0 + LC].reshape(nvalid * B, D)
        hid_c[r0 : r0 + nvalid * B] = hseg.astype(bf)
        ids_r = np.ascontiguousarray(
            ids_c.reshape(MT, 128, H).transpose(1, 0, 2).reshape(128, MT * H)
        )
        in_maps.append(
            {
                "tabs": tabs_np,
                "ids": ids_r,
                "hid": hid_c,
                "wkv": wkv_np,
                "scal": scal_np,
            }
        )
    return in_maps


def unshard_output(results):
    """results: list of per-core dicts with 'outT' [D, MOUT] -> [L, B, D]."""
    out = np.empty((L, B, D), np.float32)
    for c in range(NCORES):
        o = results[c]["outT"].astype(np.float32)
        out[c * LC : (c + 1) * LC] = o.reshape(D, LC, B).transpose(1, 2, 0)
    return out


def kernel(hidden_states, hash_input_ids, emb_tables, key_w, key_b,
           norm1_w, norm2_w, value_w, value_b, conv_w, conv_b):
    args = [hidden_states, hash_input_ids, emb_tables, key_w, key_b,
            norm1_w, norm2_w, value_w, value_b, conv_w, conv_b]
    args = [np.asarray(a) for a in args]
    in_maps = host_prep(*args)
    nc = _get_program()
    res = run_bass_kernel_spmd(nc, in_maps, list(range(NCORES)))
    return unshard_output(res.results)



# revision 61
# speedup vs baseline: 1.2642x; 1.1627x over previous
"""Engram block (hash-embedding gather + gated value + dilated causal depthwise
conv) as a Bass/Tile SPMD kernel on 8 Trainium2 NeuronCores.

Sharding: sequence (L) split 8 ways; each core recomputes a 12-position halo
for the causal conv. Embedding tables are replicated (the gather reads only
needed rows). Weights host-transposed/cast to bf16.

Per-core pipeline (per 128-token m-tile, so PE overlaps the gather):
  1. indirect-DMA gather of 12 head embeddings -> PE transpose -> embT [e, m]
  2. k|v projections as ONE matmul family: stationary = embT block (one
     LDWEIGHTS per 1024 streamed cols), moving = concat [Wk^T | Wv^T] cols;
     PSUM out is [m_tile, d_cols], so RMS/gate stats are free-dim reductions
     (ACT square-accumulate, DVE tensor_tensor_reduce) and the gate applies
     as a per-partition scalar.
  3. gated value transposed back (PE) to [d, m] for the dilated conv, which
     is 4 free-dim-shifted fused multiply-adds on DVE; fp32 result DMA'd out
     as [D, m_out] (host re-transposes when unsharding).
"""
import sys

sys.path.insert(0, "/opt/trn_rl_repo")

import numpy as np
import ml_dtypes

import concourse.bass as bass
import concourse.tile as tile
from concourse import mybir
from concourse.masks import make_identity
from concourse.bass_utils import run_bass_kernel_spmd

# problem shapes (hardcoded per spec)
L, B, D = 4096, 2, 2048
H, Dh = 12, 128
E = H * Dh  # 1536
N = 100000
K, DIL = 4, 4
EPS = 1e-6

NCORES = 8
LC = L // NCORES          # 512 l-positions per core
HALO = (K - 1) * DIL      # 12
LE = LC + HALO            # 524
M = LE * B                # 1048 valid tokens (l-major, b inner)
MP = 1152                 # padded to 9*128
MT = MP // 128            # 9 m-tiles
DT = D // 128             # 16 d-tiles
ET = E // 128             # 12 e-tiles
MOUT = LC * B             # 1024 output tokens per core
OFF = HALO * B            # 24 = first valid output token
D2 = 2 * D                # concat k|v output cols
GRP = 1024                # matmul column group (2 PSUM banks)
NGRP = D2 // GRP          # 4
# conv ranges (out-col start, width); range r ready after m-tile LAST_MT[r]
CONV_R = [(0, 488), (488, 232), (720, 256), (976, 48)]

BF16 = mybir.dt.bfloat16
F32 = mybir.dt.float32
I32 = mybir.dt.int32

# scal columns per d-tile
SC_W0, SC_W1, SC_W2, SC_W3P, SC_CB = range(5)
NSC = 5


def _split_multi_waits(nc):
    """This walrus build accepts only one sync-wait per instruction; hoist
    extra waits onto injected NOPs on the same engine (order-preserving)."""
    for f in nc.m.functions:
        for bb in f.blocks:
            new_insts = []
            for inst in bb.instructions:
                si = inst.sync_info
                if si is not None and si.on_wait and len(si.on_wait) > 1:
                    for w in si.on_wait[:-1]:
                        nop = mybir.InstNoOp(
                            name=nc.get_next_instruction_name(), ins=[], outs=[]
                        )
                        nop.engine = inst.engine
                        nop.sync_info = mybir.SyncInfo(on_wait=[w], on_update=[])
                        new_insts.append(nop)
                    si.on_wait = [si.on_wait[-1]]
                new_insts.append(inst)
            bb.instructions = new_insts


def build_program():
    nc = bass.Bass("TRN2", target_bir_lowering=False, debug=False)

    tabs = nc.declare_dram_parameter("tabs", [H * N, Dh], BF16, isOutput=False)
    ids = nc.declare_dram_parameter("ids", [128, MT * H], I32, isOutput=False)
    hid = nc.declare_dram_parameter("hid", [MP, D], BF16, isOutput=False)
    wkv = nc.declare_dram_parameter("wkv", [E, D2], BF16, isOutput=False)
    scal = nc.declare_dram_parameter("scal", [128, DT * NSC], F32, isOutput=False)
    outT = nc.declare_dram_parameter("outT", [D, MOUT], BF16, isOutput=True)

    AR = mybir.ActivationFunctionType
    ALU = mybir.AluOpType

    with tile.TileContext(nc) as tc:
        with (
            tc.tile_pool(name="persist", bufs=1) as pp,
            tc.tile_pool(name="work", bufs=3) as wp,
            tc.tile_pool(name="stat", bufs=2) as sp,
            tc.tile_pool(name="psum", bufs=2, space="PSUM") as psp,
        ):
            # ---- constants / small inputs (ids on the idle sync ring so
            #      gathers are not queued behind the 12MB weight DMAs) ----
            eps_sb = pp.tile([128, 1], F32, tag="eps")
            nc.vector.memset(eps_sb[:], EPS)

            ids_sb = pp.tile([128, MT * H], I32, tag="ids")
            nc.sync.dma_start(ids_sb[:], ids.ap())
            scal_sb = pp.tile([128, DT * NSC], F32, tag="scal")
            nc.sync.dma_start(scal_sb[:], scal.ap())

            def sc(dt_, c):
                return scal_sb[:, dt_ * NSC + c : dt_ * NSC + c + 1]

            # ---- weights (resident, concat k|v along cols) ----
            wkv_sb = []
            for e in range(ET):
                w = pp.tile([128, D2], BF16, tag=f"wkv{e}", name=f"wkv{e}")
                wkv_sb.append(w)
            for e in range(4):
                nc.scalar.dma_start(
                    wkv_sb[e][:, 0:D], wkv[e * 128 : (e + 1) * 128, 0:D]
                )
            with tc.tile_wait_until(0.010):
                for e in range(4, ET):
                    nc.scalar.dma_start(
                        wkv_sb[e][:, 0:D], wkv[e * 128 : (e + 1) * 128, 0:D]
                    )
            with tc.tile_wait_until(0.012):
                for e in range(ET):
                    nc.scalar.dma_start(
                        wkv_sb[e][:, D:D2], wkv[e * 128 : (e + 1) * 128, D:D2]
                    )

            # ---- gather all m-tiles up front (program order sets priority;
            #      Q7/SDMA stream ahead of PE consumption) ----
            bc_reg = nc.gpsimd.to_reg(H * N - 1)
            emb_raws = []
            for t in range(MT):
                er = wp.tile(
                    [128, H * Dh], BF16, tag="emb_raw", bufs=3,
                    name=f"emb_raw{t}",
                )
                if t in (0, MT - 1):
                    nc.gpsimd.memset(er[:], 0)
                for h in range(H):
                    nc.gpsimd.indirect_dma_start(
                        out=er[:, h * Dh : (h + 1) * Dh],
                        out_offset=None,
                        in_=tabs[:],
                        in_offset=bass.IndirectOffsetOnAxis(
                            ap=ids_sb[:, t * H + h : t * H + h + 1], axis=0
                        ),
                        bounds_check=bc_reg,
                        oob_is_err=False,
                    )
                emb_raws.append(er)

            ident = pp.tile([128, 128], BF16, tag="ident")
            make_identity(nc, ident[:])
            embT = [
                pp.tile([128, MP], BF16, tag=f"embT{h}", name=f"embT{h}")
                for h in range(H)
            ]
            v_sb = [
                pp.tile([128, MP], BF16, tag=f"v_sb{dt_}", name=f"v_sb{dt_}")
                for dt_ in range(DT)
            ]
            g_stats = pp.tile([128, MT], F32, tag="g_stats")  # gate G per m-tile

            def conv_range(r):
                """Emit conv + output DMA for out-col range r (all d-tiles)."""
                c0, cw = CONV_R[r]
                for dt_ in range(DT):
                    vs = v_sb[dt_]
                    a1 = wp.tile([128, cw], BF16, tag="a1", bufs=2)
                    nc.vector.tensor_scalar(
                        out=a1[:], in0=vs[:, c0 : c0 + cw],
                        scalar1=sc(dt_, SC_W0), scalar2=sc(dt_, SC_CB),
                        op0=ALU.mult, op1=ALU.add,
                    )
                    a2 = wp.tile([128, cw], BF16, tag="a2", bufs=2)
                    nc.vector.scalar_tensor_tensor(
                        out=a2[:], in0=vs[:, c0 + 8 : c0 + 8 + cw],
                        scalar=sc(dt_, SC_W1), in1=a1[:],
                        op0=ALU.mult, op1=ALU.add,
                    )
                    a3 = wp.tile([128, cw], BF16, tag="a3", bufs=2)
                    nc.vector.scalar_tensor_tensor(
                        out=a3[:], in0=vs[:, c0 + 16 : c0 + 16 + cw],
                        scalar=sc(dt_, SC_W2), in1=a2[:],
                        op0=ALU.mult, op1=ALU.add,
                    )
                    ot = wp.tile([128, cw], BF16, tag="ot", bufs=2)
                    nc.vector.scalar_tensor_tensor(
                        out=ot[:], in0=vs[:, c0 + OFF : c0 + OFF + cw],
                        scalar=sc(dt_, SC_W3P), in1=a3[:],
                        op0=ALU.mult, op1=ALU.add,
                    )
                    nc.sync.dma_start(
                        outT[dt_ * 128 : (dt_ + 1) * 128, c0 : c0 + cw], ot[:]
                    )

            def emb_tr(t_, h_):
                pt = psp.tile([128, 128], BF16, tag="tpose", space="PSUM")
                nc.tensor.transpose(
                    out=pt[:], in_=emb_raws[t_][:, h_ * Dh : (h_ + 1) * Dh],
                    identity=ident[:],
                )
                nc.scalar.copy(
                    out=embT[h_][:, t_ * 128 : (t_ + 1) * 128], in_=pt[:]
                )

            # ---- main per-m-tile pipeline. Tile t+1's emb transposes are
            #      interleaved into tile t's matmul groups so the PE<->ACT
            #      transpose/copy ping-pong hides under matmul streaming ----
            for h in range(H):
                emb_tr(0, h)
            for t in range(MT):
                # hidden rows for this m-tile (natural layout) + h^2 accum
                h_md = wp.tile([128, D], BF16, tag="h_md", bufs=2)
                nc.sync.dma_start(h_md[:], hid.ap()[t * 128 : (t + 1) * 128, :])
                sh = sp.tile([128, 1], F32, tag="sh")
                hsj = wp.tile([128, D], BF16, tag="junk", bufs=2, name="hsj")
                nc.scalar.activation(
                    out=hsj[:], in_=h_md[:], func=AR.Square, accum_out=sh[:]
                )

                # k|v matmuls in 4 col-groups of 1024 (2 PSUM banks each)
                sk_p = sp.tile([128, NGRP // 2], F32, tag="sk_p")
                pk_c = [sp.tile([128, 1], F32, tag=f"pk{i}", name=f"pk{i}_{t}")
                        for i in range(2)]
                vglo = []
                v_md = wp.tile([128, D], BF16, tag="v_md", bufs=2)
                for g in range(NGRP):
                    mm_ps = psp.tile([128, GRP], F32, tag="mm_ps", bufs=3, space="PSUM")
                    for e in range(ET):
                        for b in range(GRP // 512):
                            nc.tensor.matmul(
                                out=mm_ps[:, b * 512 : (b + 1) * 512],
                                lhsT=embT[e][:, t * 128 : (t + 1) * 128],
                                rhs=wkv_sb[e][:, g * GRP + b * 512 :
                                              g * GRP + (b + 1) * 512],
                                start=(e == 0), stop=(e == ET - 1),
                            )
                    if t + 1 < MT:
                        for h in range(3 * g, 3 * g + 3):
                            emb_tr(t + 1, h)
                    if g < 2:
                        # k stats: sum k^2 (ACT), sum k*h (DVE ttr chain)
                        ksj = wp.tile([128, GRP], BF16, tag="junk", bufs=2, name="ksj")
                        nc.scalar.activation(
                            out=ksj[:], in_=mm_ps[:], func=AR.Square,
                            accum_out=sk_p[:, g : g + 1],
                        )
                        khj = wp.tile([128, GRP], BF16, tag="junk", bufs=2, name="khj")
                        nc.vector.scalar_tensor_tensor(
                            out=khj[:], in0=mm_ps[:], scalar=1.0,
                            in1=h_md[:, g * GRP : (g + 1) * GRP],
                            op0=ALU.mult, op1=ALU.mult,
                            accum_out=pk_c[g][:],
                        )
                    else:
                        vglo.append(mm_ps)

                # gate tail for this m-tile on [128,1]
                s1 = sp.tile([128, 1], F32, tag="s1")
                nc.scalar.activation(
                    out=s1[:], in_=sk_p[:, 0:1], func=AR.Identity,
                    bias=eps_sb[:, 0:1], scale=1.0 / D,
                )
                # add second k^2 part: s1 += sk_p[:,1]/D  (fold via stt)
                s1b = sp.tile([128, 1], F32, tag="s1b")
                nc.vector.scalar_tensor_tensor(
                    out=s1b[:], in0=sk_p[:, 1:2], scalar=1.0 / D, in1=s1[:],
                    op0=ALU.mult, op1=ALU.add,
                )
                s2 = sp.tile([128, 1], F32, tag="s2")
                nc.scalar.activation(
                    out=s2[:], in_=sh[:], func=AR.Identity,
                    bias=eps_sb[:, 0:1], scale=1.0 / D,
                )
                tt = sp.tile([128, 1], F32, tag="tt")
                nc.vector.tensor_mul(tt[:], s1b[:], s2[:])
                rr = sp.tile([128, 1], F32, tag="rr")
                nc.vector.reciprocal(rr[:], tt[:])
                rq = sp.tile([128, 1], F32, tag="rq")
                nc.scalar.activation(out=rq[:], in_=rr[:], func=AR.Sqrt)
                pks = sp.tile([128, 1], F32, tag="pks")
                nc.vector.tensor_add(pks[:], pk_c[0][:], pk_c[1][:])
                uu = sp.tile([128, 1], F32, tag="uu")
                nc.vector.scalar_tensor_tensor(
                    out=uu[:], in0=pks[:], scalar=float(1.0 / np.sqrt(D)),
                    in1=rq[:], op0=ALU.mult, op1=ALU.mult,
                )
                ab = sp.tile([128, 1], F32, tag="ab")
                nc.scalar.activation(out=ab[:], in_=uu[:], func=AR.Abs)
                mx = sp.tile([128, 1], F32, tag="mx")
                nc.vector.tensor_scalar_max(out=mx[:], in0=ab[:], scalar1=1e-6)
                r2 = sp.tile([128, 1], F32, tag="r2")
                nc.vector.reciprocal(r2[:], mx[:])
                q2 = sp.tile([128, 1], F32, tag="q2")
                nc.scalar.activation(out=q2[:], in_=r2[:], func=AR.Sqrt)
                st = sp.tile([128, 1], F32, tag="st")
                nc.vector.tensor_mul(st[:], uu[:], q2[:])
                nc.scalar.activation(
                    out=g_stats[:, t : t + 1], in_=st[:], func=AR.Sigmoid
                )

                # gated value -> v_md [m, d] bf16, transposed per group so
                # the PE transposes interleave with later matmul groups
                for gi, vp in enumerate(vglo):
                    nc.vector.tensor_scalar_mul(
                        out=v_md[:, gi * GRP : (gi + 1) * GRP], in0=vp[:],
                        scalar1=g_stats[:, t : t + 1],
                    )
                    for dt_ in range(gi * 8, (gi + 1) * 8):
                        pt = psp.tile([128, 128], BF16, tag="tpose", space="PSUM")
                        nc.tensor.transpose(
                            out=pt[:], in_=v_md[:, dt_ * 128 : (dt_ + 1) * 128],
                            identity=ident[:],
                        )
                        nc.scalar.copy(
                            out=v_sb[dt_][:, t * 128 : (t + 1) * 128], in_=pt[:]
                        )

                if t == 4:
                    conv_range(0)
                if t == 6:
                    conv_range(1)
                if t == 8:
                    conv_range(2)
            conv_range(3)

    _split_multi_waits(nc)
    return nc


_CACHE = {}


def _get_program():
    if "nc" not in _CACHE:
        _CACHE["nc"] = build_program()
    return _CACHE["nc"]


def host_prep(hidden_states, hash_input_ids, emb_tables, key_w, key_b,
              norm1_w, norm2_w, value_w, value_b, conv_w, conv_b):
    """Shard + lay out inputs for the 8 cores. Returns in_maps list."""
    bf = ml_dtypes.bfloat16
    w12 = norm1_w.astype(np.float64) * norm2_w.astype(np.float64)
    assert np.allclose(w12, 1.0, atol=1e-5), (
        "fast path assumes norm1_w*norm2_w == 1 (problem spec: fill=ones)"
    )
    assert not key_b.any() and not value_b.any(), (
        "fast path assumes zero key/value biases (problem spec: fill=zeros)"
    )

    tabs_np = np.ascontiguousarray(emb_tables.reshape(H * N, Dh)).astype(bf)
    wkv_np = np.empty((E, D2), bf)
    wkv_np[:, :D] = key_w.T.astype(bf)
    wkv_np[:, D:] = value_w.T.astype(bf)
    scal_d = np.empty((D, NSC), np.float32)
    scal_d[:, SC_W0] = conv_w[:, 0]
    scal_d[:, SC_W1] = conv_w[:, 1]
    scal_d[:, SC_W2] = conv_w[:, 2]
    scal_d[:, SC_W3P] = conv_w[:, 3] + 1.0
    scal_d[:, SC_CB] = conv_b
    scal_np = np.ascontiguousarray(
        scal_d.reshape(DT, 128, NSC).transpose(1, 0, 2).reshape(128, DT * NSC)
    )

    head_off = (np.arange(H, dtype=np.int64) * N)[None, :]
    OOB = np.int32(H * N)

    in_maps = []
    for c in range(NCORES):
        l0 = c * LC
        lo = l0 - HALO
        lo_clip = max(lo, 0)
        nvalid = (l0 + LC) - lo_clip
        r0 = (lo_clip - lo) * B
        ids_c = np.full((MP, H), OOB, np.int32)
        seg = hash_input_ids[lo_clip : l0 + LC].reshape(nvalid * B, H)
        ids_c[r0 : r0 + nvalid * B] = (seg.astype(np.int64) + head_off).astype(
            np.int32
        )
        hid_c = np.zeros((MP, D), bf)
        hseg = hidden_states[lo_clip : l0 + LC].reshape(nvalid * B, D)
        hid_c[r0 : r0 + nvalid * B] = hseg.astype(bf)
        ids_r = np.ascontiguousarray(
            ids_c.reshape(MT, 128, H).transpose(1, 0, 2).reshape(128, MT * H)
        )
        in_maps.append(
            {
                "tabs": tabs_np,
                "ids": ids_r,
                "hid": hid_c,
                "wkv": wkv_np,
                "scal": scal_np,
            }
        )
    return in_maps


def unshard_output(results):
    """results: list of per-core dicts with 'outT' [D, MOUT] -> [L, B, D]."""
    out = np.empty((L, B, D), np.float32)
    for c in range(NCORES):
        o = results[c]["outT"].astype(np.float32)
        out[c * LC : (c + 1) * LC] = o.reshape(D, LC, B).transpose(1, 2, 0)
    return out


def kernel(hidden_states, hash_input_ids, emb_tables, key_w, key_b,
           norm1_w, norm2_w, value_w, value_b, conv_w, conv_b):
    args = [hidden_states, hash_input_ids, emb_tables, key_w, key_b,
            norm1_w, norm2_w, value_w, value_b, conv_w, conv_b]
    args = [np.asarray(a) for a in args]
    in_maps = host_prep(*args)
    nc = _get_program()
    res = run_bass_kernel_spmd(nc, in_maps, list(range(NCORES)))
    return unshard_output(res.results)



# revision 64
# speedup vs baseline: 1.3302x; 1.0522x over previous
"""Engram block (hash-embedding gather + gated value + dilated causal depthwise
conv) as a Bass/Tile SPMD kernel on 8 Trainium2 NeuronCores.

Sharding: sequence (L) split 8 ways; each core recomputes a 12-position halo
for the causal conv. Embedding tables are replicated (the gather reads only
needed rows). Weights host-transposed/cast to bf16.

Per-core pipeline (per 128-token m-tile, so PE overlaps the gather):
  1. indirect-DMA gather of 12 head embeddings -> PE transpose -> embT [e, m]
  2. k|v projections as ONE matmul family: stationary = embT block (one
     LDWEIGHTS per 1024 streamed cols), moving = concat [Wk^T | Wv^T] cols;
     PSUM out is [m_tile, d_cols], so RMS/gate stats are free-dim reductions
     (ACT square-accumulate, DVE tensor_tensor_reduce) and the gate applies
     as a per-partition scalar.
  3. gated value transposed back (PE) to [d, m] for the dilated conv, which
     is 4 free-dim-shifted fused multiply-adds on DVE; fp32 result DMA'd out
     as [D, m_out] (host re-transposes when unsharding).
"""
import sys

sys.path.insert(0, "/opt/trn_rl_repo")

import numpy as np
import ml_dtypes

import concourse.bass as bass
import concourse.tile as tile
from concourse import mybir
from concourse.masks import make_identity
from concourse.bass_utils import run_bass_kernel_spmd

# problem shapes (hardcoded per spec)
L, B, D = 4096, 2, 2048
H, Dh = 12, 128
E = H * Dh  # 1536
N = 100000
K, DIL = 4, 4
EPS = 1e-6

NCORES = 8
LC = L // NCORES          # 512 l-positions per core
HALO = (K - 1) * DIL      # 12
LE = LC + HALO            # 524
M = LE * B                # 1048 valid tokens (l-major, b inner)
MP = 1152                 # padded to 9*128
MT = MP // 128            # 9 m-tiles
DT = D // 128             # 16 d-tiles
ET = E // 128             # 12 e-tiles
MOUT = LC * B             # 1024 output tokens per core
OFF = HALO * B            # 24 = first valid output token
D2 = 2 * D                # concat k|v output cols
GRP = 1024                # matmul column group (2 PSUM banks)
NGRP = D2 // GRP          # 4
# conv ranges (out-col start, width); range r ready after m-tile LAST_MT[r]
CONV_R = [(0, 488), (488, 232), (720, 256), (976, 48)]

BF16 = mybir.dt.bfloat16
F32 = mybir.dt.float32
I32 = mybir.dt.int32

# scal columns per d-tile
SC_W0, SC_W1, SC_W2, SC_W3P, SC_CB = range(5)
NSC = 5


def _split_multi_waits(nc):
    """This walrus build accepts only one sync-wait per instruction; hoist
    extra waits onto injected NOPs on the same engine (order-preserving)."""
    for f in nc.m.functions:
        for bb in f.blocks:
            new_insts = []
            for inst in bb.instructions:
                si = inst.sync_info
                if si is not None and si.on_wait and len(si.on_wait) > 1:
                    for w in si.on_wait[:-1]:
                        nop = mybir.InstNoOp(
                            name=nc.get_next_instruction_name(), ins=[], outs=[]
                        )
                        nop.engine = inst.engine
                        nop.sync_info = mybir.SyncInfo(on_wait=[w], on_update=[])
                        new_insts.append(nop)
                    si.on_wait = [si.on_wait[-1]]
                new_insts.append(inst)
            bb.instructions = new_insts


def build_program():
    nc = bass.Bass("TRN2", target_bir_lowering=False, debug=False)

    tabs = nc.declare_dram_parameter("tabs", [H * N, Dh], BF16, isOutput=False)
    ids = nc.declare_dram_parameter("ids", [128, MT * H], I32, isOutput=False)
    hid = nc.declare_dram_parameter("hid", [MP, D], BF16, isOutput=False)
    wkv = nc.declare_dram_parameter("wkv", [E, D2], BF16, isOutput=False)
    scal = nc.declare_dram_parameter("scal", [128, DT * NSC], F32, isOutput=False)
    outT = nc.declare_dram_parameter("outT", [D, MOUT], BF16, isOutput=True)

    AR = mybir.ActivationFunctionType
    ALU = mybir.AluOpType

    with tile.TileContext(nc) as tc:
        with (
            tc.tile_pool(name="persist", bufs=1) as pp,
            tc.tile_pool(name="work", bufs=3) as wp,
            tc.tile_pool(name="stat", bufs=2) as sp,
            tc.tile_pool(name="psum", bufs=2, space="PSUM") as psp,
        ):
            # ---- constants / small inputs (ids on the idle sync ring so
            #      gathers are not queued behind the 12MB weight DMAs) ----
            eps_sb = pp.tile([128, 1], F32, tag="eps")
            nc.vector.memset(eps_sb[:], EPS)

            ids_sb = pp.tile([128, MT * H], I32, tag="ids")
            nc.sync.dma_start(ids_sb[:], ids.ap())
            scal_sb = pp.tile([128, DT * NSC], F32, tag="scal")
            nc.sync.dma_start(scal_sb[:], scal.ap())

            def sc(dt_, c):
                return scal_sb[:, dt_ * NSC + c : dt_ * NSC + c + 1]

            # ---- weights (resident, concat k|v along cols) ----
            wkv_sb = []
            for e in range(ET):
                w = pp.tile([128, D2], BF16, tag=f"wkv{e}", name=f"wkv{e}")
                wkv_sb.append(w)
            for e in range(4):
                nc.scalar.dma_start(
                    wkv_sb[e][:, 0:D], wkv[e * 128 : (e + 1) * 128, 0:D]
                )
            with tc.tile_wait_until(0.010):
                for e in range(4, ET):
                    nc.scalar.dma_start(
                        wkv_sb[e][:, 0:D], wkv[e * 128 : (e + 1) * 128, 0:D]
                    )
            with tc.tile_wait_until(0.012):
                for e in range(ET):
                    nc.scalar.dma_start(
                        wkv_sb[e][:, D:D2], wkv[e * 128 : (e + 1) * 128, D:D2]
                    )

            # ---- gather all m-tiles up front (program order sets priority;
            #      Q7/SDMA stream ahead of PE consumption) ----
            bc_reg = nc.gpsimd.to_reg(H * N - 1)
            emb_raws = []
            for t in range(MT):
                er = wp.tile(
                    [128, H * Dh], BF16, tag="emb_raw", bufs=3,
                    name=f"emb_raw{t}",
                )
                if t in (0, MT - 1):
                    nc.gpsimd.memset(er[:], 0)
                for h in range(H):
                    nc.gpsimd.indirect_dma_start(
                        out=er[:, h * Dh : (h + 1) * Dh],
                        out_offset=None,
                        in_=tabs[:],
                        in_offset=bass.IndirectOffsetOnAxis(
                            ap=ids_sb[:, t * H + h : t * H + h + 1], axis=0
                        ),
                        bounds_check=bc_reg,
                        oob_is_err=False,
                    )
                emb_raws.append(er)

            ident = pp.tile([128, 128], BF16, tag="ident")
            make_identity(nc, ident[:])
            embT = [
                pp.tile([128, MP], BF16, tag=f"embT{h}", name=f"embT{h}")
                for h in range(H)
            ]
            v_sb = [
                pp.tile([128, MP], BF16, tag=f"v_sb{dt_}", name=f"v_sb{dt_}")
                for dt_ in range(DT)
            ]
            g_stats = pp.tile([128, MT], F32, tag="g_stats")  # gate G per m-tile

            def conv_range(r, dts=None):
                """Emit conv + output DMA for out-col range r."""
                c0, cw = CONV_R[r]
                for dt_ in (range(DT) if dts is None else dts):
                    vs = v_sb[dt_]
                    a1 = wp.tile([128, cw], BF16, tag="a1", bufs=2)
                    nc.vector.tensor_scalar(
                        out=a1[:], in0=vs[:, c0 : c0 + cw],
                        scalar1=sc(dt_, SC_W0), scalar2=sc(dt_, SC_CB),
                        op0=ALU.mult, op1=ALU.add,
                    )
                    a2 = wp.tile([128, cw], BF16, tag="a2", bufs=2)
                    nc.vector.scalar_tensor_tensor(
                        out=a2[:], in0=vs[:, c0 + 8 : c0 + 8 + cw],
                        scalar=sc(dt_, SC_W1), in1=a1[:],
                        op0=ALU.mult, op1=ALU.add,
                    )
                    a3 = wp.tile([128, cw], BF16, tag="a3", bufs=2)
                    nc.vector.scalar_tensor_tensor(
                        out=a3[:], in0=vs[:, c0 + 16 : c0 + 16 + cw],
                        scalar=sc(dt_, SC_W2), in1=a2[:],
                        op0=ALU.mult, op1=ALU.add,
                    )
                    ot = wp.tile([128, cw], BF16, tag="ot", bufs=2)
                    nc.vector.scalar_tensor_tensor(
                        out=ot[:], in0=vs[:, c0 + OFF : c0 + OFF + cw],
                        scalar=sc(dt_, SC_W3P), in1=a3[:],
                        op0=ALU.mult, op1=ALU.add,
                    )
                    nc.sync.dma_start(
                        outT[dt_ * 128 : (dt_ + 1) * 128, c0 : c0 + cw], ot[:]
                    )

            def emb_tr(t_, h_):
                pt = psp.tile([128, 128], BF16, tag="tpose", space="PSUM")
                nc.tensor.transpose(
                    out=pt[:], in_=emb_raws[t_][:, h_ * Dh : (h_ + 1) * Dh],
                    identity=ident[:],
                )
                nc.scalar.copy(
                    out=embT[h_][:, t_ * 128 : (t_ + 1) * 128], in_=pt[:]
                )

            # ---- main per-m-tile pipeline. Tile t+1's emb transposes are
            #      interleaved into tile t's matmul groups so the PE<->ACT
            #      transpose/copy ping-pong hides under matmul streaming ----
            for h in range(H):
                emb_tr(0, h)
            for t in range(MT):
                # hidden rows for this m-tile (natural layout) + h^2 accum
                h_md = wp.tile([128, D], BF16, tag="h_md", bufs=2)
                nc.sync.dma_start(h_md[:], hid.ap()[t * 128 : (t + 1) * 128, :])
                sh = sp.tile([128, 1], F32, tag="sh")
                hsj = wp.tile([128, D], BF16, tag="junk", bufs=2, name="hsj")
                nc.scalar.activation(
                    out=hsj[:], in_=h_md[:], func=AR.Square, accum_out=sh[:]
                )

                # k|v matmuls in 4 col-groups of 1024 (2 PSUM banks each)
                sk_p = sp.tile([128, NGRP // 2], F32, tag="sk_p")
                pk_c = [sp.tile([128, 1], F32, tag=f"pk{i}", name=f"pk{i}_{t}")
                        for i in range(2)]
                vglo = []
                v_md = wp.tile([128, D], BF16, tag="v_md", bufs=2)
                for g in range(NGRP):
                    mm_ps = psp.tile([128, GRP], F32, tag="mm_ps", bufs=3, space="PSUM")
                    for e in range(ET):
                        for b in range(GRP // 512):
                            nc.tensor.matmul(
                                out=mm_ps[:, b * 512 : (b + 1) * 512],
                                lhsT=embT[e][:, t * 128 : (t + 1) * 128],
                                rhs=wkv_sb[e][:, g * GRP + b * 512 :
                                              g * GRP + (b + 1) * 512],
                                start=(e == 0), stop=(e == ET - 1),
                            )
                    if t + 1 < MT:
                        for h in range(3 * g, 3 * g + 3):
                            emb_tr(t + 1, h)
                    if g < 2:
                        # k stats: sum k^2 (ACT), sum k*h (DVE ttr chain)
                        ksj = wp.tile([128, GRP], BF16, tag="junk", bufs=2, name="ksj")
                        nc.scalar.activation(
                            out=ksj[:], in_=mm_ps[:], func=AR.Square,
                            accum_out=sk_p[:, g : g + 1],
                        )
                        khj = wp.tile([128, GRP], BF16, tag="junk", bufs=2, name="khj")
                        nc.vector.scalar_tensor_tensor(
                            out=khj[:], in0=mm_ps[:], scalar=1.0,
                            in1=h_md[:, g * GRP : (g + 1) * GRP],
                            op0=ALU.mult, op1=ALU.mult,
                            accum_out=pk_c[g][:],
                        )
                    else:
                        vglo.append(mm_ps)

                # gate tail for this m-tile on [128,1]
                s1 = sp.tile([128, 1], F32, tag="s1")
                nc.scalar.activation(
                    out=s1[:], in_=sk_p[:, 0:1], func=AR.Identity,
                    bias=eps_sb[:, 0:1], scale=1.0 / D,
                )
                # add second k^2 part: s1 += sk_p[:,1]/D  (fold via stt)
                s1b = sp.tile([128, 1], F32, tag="s1b")
                nc.vector.scalar_tensor_tensor(
                    out=s1b[:], in0=sk_p[:, 1:2], scalar=1.0 / D, in1=s1[:],
                    op0=ALU.mult, op1=ALU.add,
                )
                s2 = sp.tile([128, 1], F32, tag="s2")
                nc.scalar.activation(
                    out=s2[:], in_=sh[:], func=AR.Identity,
                    bias=eps_sb[:, 0:1], scale=1.0 / D,
                )
                tt = sp.tile([128, 1], F32, tag="tt")
                nc.vector.tensor_mul(tt[:], s1b[:], s2[:])
                rr = sp.tile([128, 1], F32, tag="rr")
                nc.vector.reciprocal(rr[:], tt[:])
                rq = sp.tile([128, 1], F32, tag="rq")
                nc.scalar.activation(out=rq[:], in_=rr[:], func=AR.Sqrt)
                pks = sp.tile([128, 1], F32, tag="pks")
                nc.vector.tensor_add(pks[:], pk_c[0][:], pk_c[1][:])
                uu = sp.tile([128, 1], F32, tag="uu")
                nc.vector.scalar_tensor_tensor(
                    out=uu[:], in0=pks[:], scalar=float(1.0 / np.sqrt(D)),
                    in1=rq[:], op0=ALU.mult, op1=ALU.mult,
                )
                ab = sp.tile([128, 1], F32, tag="ab")
                nc.scalar.activation(out=ab[:], in_=uu[:], func=AR.Abs)
                mx = sp.tile([128, 1], F32, tag="mx")
                nc.vector.tensor_scalar_max(out=mx[:], in0=ab[:], scalar1=1e-6)
                r2 = sp.tile([128, 1], F32, tag="r2")
                nc.vector.reciprocal(r2[:], mx[:])
                q2 = sp.tile([128, 1], F32, tag="q2")
                nc.scalar.activation(out=q2[:], in_=r2[:], func=AR.Sqrt)
                st = sp.tile([128, 1], F32, tag="st")
                nc.vector.tensor_mul(st[:], uu[:], q2[:])
                nc.scalar.activation(
                    out=g_stats[:, t : t + 1], in_=st[:], func=AR.Sigmoid
                )

                # gated value -> v_md [m, d] bf16, transposed per group so
                # the PE transposes interleave with later matmul groups
                for gi, vp in enumerate(vglo):
                    nc.vector.tensor_scalar_mul(
                        out=v_md[:, gi * GRP : (gi + 1) * GRP], in0=vp[:],
                        scalar1=g_stats[:, t : t + 1],
                    )
                    for dt_ in range(gi * 8, (gi + 1) * 8):
                        pt = psp.tile([128, 128], BF16, tag="tpose", space="PSUM")
                        nc.tensor.transpose(
                            out=pt[:], in_=v_md[:, dt_ * 128 : (dt_ + 1) * 128],
                            identity=ident[:],
                        )
                        # last tile: only 24 token cols are live
                        cw_t = 128 if t < MT - 1 else 24
                        nc.scalar.copy(
                            out=v_sb[dt_][:, t * 128 : t * 128 + cw_t],
                            in_=pt[:, 0:cw_t],
                        )

                if t == 4:
                    conv_range(0)
                if t == 6:
                    conv_range(1)
                if t == 8:
                    conv_range(2)

            # final range: accumulate all 16 d-tiles into one tile and ship
            # a single 3D-AP DMA instead of 16 tiny ones (dispatch-bound)
            c0f, cwf = CONV_R[3]
            otb = wp.tile([128, DT, 48], BF16, tag="otb")
            for dt_ in range(DT):
                vs = v_sb[dt_]
                a1 = wp.tile([128, cwf], BF16, tag="a1", bufs=2)
                nc.vector.tensor_scalar(
                    out=a1[:], in0=vs[:, c0f : c0f + cwf],
                    scalar1=sc(dt_, SC_W0), scalar2=sc(dt_, SC_CB),
                    op0=ALU.mult, op1=ALU.add,
                )
                a2 = wp.tile([128, cwf], BF16, tag="a2", bufs=2)
                nc.vector.scalar_tensor_tensor(
                    out=a2[:], in0=vs[:, c0f + 8 : c0f + 8 + cwf],
                    scalar=sc(dt_, SC_W1), in1=a1[:],
                    op0=ALU.mult, op1=ALU.add,
                )
                a3 = wp.tile([128, cwf], BF16, tag="a3", bufs=2)
                nc.vector.scalar_tensor_tensor(
                    out=a3[:], in0=vs[:, c0f + 16 : c0f + 16 + cwf],
                    scalar=sc(dt_, SC_W2), in1=a2[:],
                    op0=ALU.mult, op1=ALU.add,
                )
                nc.vector.scalar_tensor_tensor(
                    out=otb[:, dt_, :], in0=vs[:, c0f + OFF : c0f + OFF + cwf],
                    scalar=sc(dt_, SC_W3P), in1=a3[:],
                    op0=ALU.mult, op1=ALU.add,
                )
            nc.sync.dma_start(
                outT.ap()[:, c0f : c0f + cwf].rearrange(
                    "(dt p) x -> p dt x", p=128
                ),
                otb[:, :, :],
            )

    _split_multi_waits(nc)
    return nc


_CACHE = {}


def _get_program():
    if "nc" not in _CACHE:
        _CACHE["nc"] = build_program()
    return _CACHE["nc"]


def host_prep(hidden_states, hash_input_ids, emb_tables, key_w, key_b,
              norm1_w, norm2_w, value_w, value_b, conv_w, conv_b):
    """Shard + lay out inputs for the 8 cores. Returns in_maps list."""
    bf = ml_dtypes.bfloat16
    w12 = norm1_w.astype(np.float64) * norm2_w.astype(np.float64)
    assert np.allclose(w12, 1.0, atol=1e-5), (
        "fast path assumes norm1_w*norm2_w == 1 (problem spec: fill=ones)"
    )
    assert not key_b.any() and not value_b.any(), (
        "fast path assumes zero key/value biases (problem spec: fill=zeros)"
    )

    tabs_np = np.ascontiguousarray(emb_tables.reshape(H * N, Dh)).astype(bf)
    wkv_np = np.empty((E, D2), bf)
    wkv_np[:, :D] = key_w.T.astype(bf)
    wkv_np[:, D:] = value_w.T.astype(bf)
    scal_d = np.empty((D, NSC), np.float32)
    scal_d[:, SC_W0] = conv_w[:, 0]
    scal_d[:, SC_W1] = conv_w[:, 1]
    scal_d[:, SC_W2] = conv_w[:, 2]
    scal_d[:, SC_W3P] = conv_w[:, 3] + 1.0
    scal_d[:, SC_CB] = conv_b
    scal_np = np.ascontiguousarray(
        scal_d.reshape(DT, 128, NSC).transpose(1, 0, 2).reshape(128, DT * NSC)
    )

    head_off = (np.arange(H, dtype=np.int64) * N)[None, :]
    OOB = np.int32(H * N)

    in_maps = []
    for c in range(NCORES):
        l0 = c * LC
        lo = l0 - HALO
        lo_clip = max(lo, 0)
        nvalid = (l0 + LC) - lo_clip
        r0 = (lo_clip - lo) * B
        ids_c = np.full((MP, H), OOB, np.int32)
        seg = hash_input_ids[lo_clip : l0 + LC].reshape(nvalid * B, H)
        ids_c[r0 : r0 + nvalid * B] = (seg.astype(np.int64) + head_off).astype(
            np.int32
        )
        hid_c = np.zeros((MP, D), bf)
        hseg = hidden_states[lo_clip : l0 + LC].reshape(nvalid * B, D)
        hid_c[r0 : r0 + nvalid * B] = hseg.astype(bf)
        ids_r = np.ascontiguousarray(
            ids_c.reshape(MT, 128, H).transpose(1, 0, 2).reshape(128, MT * H)
        )
        in_maps.append(
            {
                "tabs": tabs_np,
                "ids": ids_r,
                "hid": hid_c,
                "wkv": wkv_np,
                "scal": scal_np,
            }
        )
    return in_maps


def unshard_output(results):
    """results: list of per-core dicts with 'outT' [D, MOUT] -> [L, B, D]."""
    out = np.empty((L, B, D), np.float32)
    for c in range(NCORES):
        o = results[c]["outT"].astype(np.float32)
        out[c * LC : (c + 1) * LC] = o.reshape(D, LC, B).transpose(1, 2, 0)
    return out


def kernel(hidden_states, hash_input_ids, emb_tables, key_w, key_b,
           norm1_w, norm2_w, value_w, value_b, conv_w, conv_b):
    args = [hidden_states, hash_input_ids, emb_tables, key_w, key_b,
            norm1_w, norm2_w, value_w, value_b, conv_w, conv_b]
    args = [np.asarray(a) for a in args]
    in_maps = host_prep(*args)
    nc = _get_program()
    res = run_bass_kernel_spmd(nc, in_maps, list(range(NCORES)))
    return unshard_output(res.results)



# revision 67
# speedup vs baseline: 1.3329x; 1.0020x over previous
"""Engram block (hash-embedding gather + gated value + dilated causal depthwise
conv) as a Bass/Tile SPMD kernel on 8 Trainium2 NeuronCores.

Sharding: sequence (L) split 8 ways; each core recomputes a 12-position halo
for the causal conv. Embedding tables are replicated (the gather reads only
needed rows). Weights host-transposed/cast to bf16.

Per-core pipeline (per 128-token m-tile, so PE overlaps the gather):
  1. indirect-DMA gather of 12 head embeddings -> PE transpose -> embT [e, m]
  2. k|v projections as ONE matmul family: stationary = embT block (one
     LDWEIGHTS per 1024 streamed cols), moving = concat [Wk^T | Wv^T] cols;
     PSUM out is [m_tile, d_cols], so RMS/gate stats are free-dim reductions
     (ACT square-accumulate, DVE tensor_tensor_reduce) and the gate applies
     as a per-partition scalar.
  3. gated value transposed back (PE) to [d, m] for the dilated conv, which
     is 4 free-dim-shifted fused multiply-adds on DVE; fp32 result DMA'd out
     as [D, m_out] (host re-transposes when unsharding).
"""
import sys

sys.path.insert(0, "/opt/trn_rl_repo")

import numpy as np
import ml_dtypes

import concourse.bass as bass
import concourse.tile as tile
from concourse import mybir
from concourse.masks import make_identity
from concourse.bass_utils import run_bass_kernel_spmd

# problem shapes (hardcoded per spec)
L, B, D = 4096, 2, 2048
H, Dh = 12, 128
E = H * Dh  # 1536
N = 100000
K, DIL = 4, 4
EPS = 1e-6

NCORES = 8
LC = L // NCORES          # 512 l-positions per core
HALO = (K - 1) * DIL      # 12
LE = LC + HALO            # 524
M = LE * B                # 1048 valid tokens (l-major, b inner)
MP = 1152                 # padded to 9*128
MT = MP // 128            # 9 m-tiles
DT = D // 128             # 16 d-tiles
ET = E // 128             # 12 e-tiles
MOUT = LC * B             # 1024 output tokens per core
OFF = HALO * B            # 24 = first valid output token
D2 = 2 * D                # concat k|v output cols
GRP = 1024                # matmul column group (2 PSUM banks)
NGRP = D2 // GRP          # 4
# conv ranges (out-col start, width); range r ready after m-tile LAST_MT[r]
CONV_R = [(0, 488), (488, 232), (720, 256), (976, 48)]

BF16 = mybir.dt.bfloat16
F32 = mybir.dt.float32
I32 = mybir.dt.int32

# scal columns per d-tile
SC_W0, SC_W1, SC_W2, SC_W3P, SC_CB = range(5)
NSC = 5


def _split_multi_waits(nc):
    """This walrus build accepts only one sync-wait per instruction; hoist
    extra waits onto injected NOPs on the same engine (order-preserving)."""
    for f in nc.m.functions:
        for bb in f.blocks:
            new_insts = []
            for inst in bb.instructions:
                si = inst.sync_info
                if si is not None and si.on_wait and len(si.on_wait) > 1:
                    for w in si.on_wait[:-1]:
                        nop = mybir.InstNoOp(
                            name=nc.get_next_instruction_name(), ins=[], outs=[]
                        )
                        nop.engine = inst.engine
                        nop.sync_info = mybir.SyncInfo(on_wait=[w], on_update=[])
                        new_insts.append(nop)
                    si.on_wait = [si.on_wait[-1]]
                new_insts.append(inst)
            bb.instructions = new_insts


def build_program():
    nc = bass.Bass("TRN2", target_bir_lowering=False, debug=False)

    tabs = nc.declare_dram_parameter("tabs", [H * N, Dh], BF16, isOutput=False)
    ids = nc.declare_dram_parameter("ids", [128, MT * H], I32, isOutput=False)
    hid = nc.declare_dram_parameter("hid", [MP, D], BF16, isOutput=False)
    wkv = nc.declare_dram_parameter("wkv", [E, D2], BF16, isOutput=False)
    scal = nc.declare_dram_parameter("scal", [128, DT * NSC], F32, isOutput=False)
    outT = nc.declare_dram_parameter("outT", [D, MOUT], BF16, isOutput=True)

    AR = mybir.ActivationFunctionType
    ALU = mybir.AluOpType

    with tile.TileContext(nc) as tc:
        with (
            tc.tile_pool(name="persist", bufs=1) as pp,
            tc.tile_pool(name="work", bufs=3) as wp,
            tc.tile_pool(name="stat", bufs=2) as sp,
            tc.tile_pool(name="psum", bufs=2, space="PSUM") as psp,
        ):
            # ---- constants / small inputs (ids on the idle sync ring so
            #      gathers are not queued behind the 12MB weight DMAs) ----
            eps_sb = pp.tile([128, 1], F32, tag="eps")
            nc.vector.memset(eps_sb[:], EPS)

            ids_sb = pp.tile([128, MT * H], I32, tag="ids")
            nc.sync.dma_start(ids_sb[:], ids.ap())
            scal_sb = pp.tile([128, DT * NSC], F32, tag="scal")
            nc.sync.dma_start(scal_sb[:], scal.ap())

            def sc(dt_, c):
                return scal_sb[:, dt_ * NSC + c : dt_ * NSC + c + 1]

            # ---- weights (resident, concat k|v along cols) ----
            wkv_sb = []
            for e in range(ET):
                w = pp.tile([128, D2], BF16, tag=f"wkv{e}", name=f"wkv{e}")
                wkv_sb.append(w)
            for e in range(4):
                nc.scalar.dma_start(
                    wkv_sb[e][:, 0:D], wkv[e * 128 : (e + 1) * 128, 0:D]
                )
            with tc.tile_wait_until(0.010):
                for e in range(4, ET):
                    nc.scalar.dma_start(
                        wkv_sb[e][:, 0:D], wkv[e * 128 : (e + 1) * 128, 0:D]
                    )
            with tc.tile_wait_until(0.012):
                for e in range(ET):
                    nc.scalar.dma_start(
                        wkv_sb[e][:, D:D2], wkv[e * 128 : (e + 1) * 128, D:D2]
                    )

            # ---- gather all m-tiles up front (program order sets priority;
            #      Q7/SDMA stream ahead of PE consumption) ----
            bc_reg = nc.gpsimd.to_reg(H * N - 1)
            emb_raws = []
            for t in range(MT):
                er = wp.tile(
                    [128, H * Dh], BF16, tag="emb_raw", bufs=3,
                    name=f"emb_raw{t}",
                )
                if t in (0, MT - 1):
                    nc.gpsimd.memset(er[:], 0)
                for h in range(H):
                    nc.gpsimd.indirect_dma_start(
                        out=er[:, h * Dh : (h + 1) * Dh],
                        out_offset=None,
                        in_=tabs[:],
                        in_offset=bass.IndirectOffsetOnAxis(
                            ap=ids_sb[:, t * H + h : t * H + h + 1], axis=0
                        ),
                        bounds_check=bc_reg,
                        oob_is_err=False,
                    )
                emb_raws.append(er)

            ident = pp.tile([128, 128], BF16, tag="ident")
            make_identity(nc, ident[:])
            embT = [
                pp.tile([128, MP], BF16, tag=f"embT{h}", name=f"embT{h}")
                for h in range(H)
            ]
            v_sb = [
                pp.tile([128, MP], BF16, tag=f"v_sb{dt_}", name=f"v_sb{dt_}")
                for dt_ in range(DT)
            ]
            g_stats = pp.tile([128, MT], F32, tag="g_stats")  # gate G per m-tile

            def conv_range(r, dts=None):
                """Emit conv + output DMA for out-col range r."""
                c0, cw = CONV_R[r]
                for dt_ in (range(DT) if dts is None else dts):
                    vs = v_sb[dt_]
                    a1 = wp.tile([128, cw], BF16, tag="a1", bufs=2)
                    nc.vector.tensor_scalar(
                        out=a1[:], in0=vs[:, c0 : c0 + cw],
                        scalar1=sc(dt_, SC_W0), scalar2=sc(dt_, SC_CB),
                        op0=ALU.mult, op1=ALU.add,
                    )
                    a2 = wp.tile([128, cw], BF16, tag="a2", bufs=2)
                    nc.vector.scalar_tensor_tensor(
                        out=a2[:], in0=vs[:, c0 + 8 : c0 + 8 + cw],
                        scalar=sc(dt_, SC_W1), in1=a1[:],
                        op0=ALU.mult, op1=ALU.add,
                    )
                    a3 = wp.tile([128, cw], BF16, tag="a3", bufs=2)
                    nc.vector.scalar_tensor_tensor(
                        out=a3[:], in0=vs[:, c0 + 16 : c0 + 16 + cw],
                        scalar=sc(dt_, SC_W2), in1=a2[:],
                        op0=ALU.mult, op1=ALU.add,
                    )
                    ot = wp.tile([128, cw], BF16, tag="ot", bufs=2)
                    nc.vector.scalar_tensor_tensor(
                        out=ot[:], in0=vs[:, c0 + OFF : c0 + OFF + cw],
                        scalar=sc(dt_, SC_W3P), in1=a3[:],
                        op0=ALU.mult, op1=ALU.add,
                    )
                    nc.sync.dma_start(
                        outT[dt_ * 128 : (dt_ + 1) * 128, c0 : c0 + cw], ot[:]
                    )

            def emb_tr(t_, h_):
                pt = psp.tile([128, 128], BF16, tag="tpose", space="PSUM")
                nc.tensor.transpose(
                    out=pt[:], in_=emb_raws[t_][:, h_ * Dh : (h_ + 1) * Dh],
                    identity=ident[:],
                )
                nc.scalar.copy(
                    out=embT[h_][:, t_ * 128 : (t_ + 1) * 128], in_=pt[:]
                )

            # ---- main per-m-tile pipeline. Tile t+1's emb transposes are
            #      interleaved into tile t's matmul groups so the PE<->ACT
            #      transpose/copy ping-pong hides under matmul streaming ----
            for h in range(H):
                emb_tr(0, h)
            for t in range(MT):
                # hidden rows for this m-tile (natural layout) + h^2 accum
                h_md = wp.tile([128, D], BF16, tag="h_md", bufs=2)
                nc.sync.dma_start(h_md[:], hid.ap()[t * 128 : (t + 1) * 128, :])
                sh = sp.tile([128, 1], F32, tag="sh")
                hsj = wp.tile([128, D], BF16, tag="junk", bufs=2, name="hsj")
                nc.scalar.activation(
                    out=hsj[:], in_=h_md[:], func=AR.Square, accum_out=sh[:]
                )

                # k|v matmuls in 4 col-groups of 1024 (2 PSUM banks each)
                sk_p = sp.tile([128, NGRP // 2], F32, tag="sk_p")
                pk_c = [sp.tile([128, 1], F32, tag=f"pk{i}", name=f"pk{i}_{t}")
                        for i in range(2)]
                vglo = []
                v_md = wp.tile([128, D], BF16, tag="v_md", bufs=2)
                for g in range(NGRP):
                    mm_ps = psp.tile([128, GRP], F32, tag="mm_ps", bufs=3, space="PSUM")
                    for e in range(ET):
                        for b in range(GRP // 512):
                            nc.tensor.matmul(
                                out=mm_ps[:, b * 512 : (b + 1) * 512],
                                lhsT=embT[e][:, t * 128 : (t + 1) * 128],
                                rhs=wkv_sb[e][:, g * GRP + b * 512 :
                                              g * GRP + (b + 1) * 512],
                                start=(e == 0), stop=(e == ET - 1),
                            )
                        if t + 1 < MT and e % 4 == 3:
                            emb_tr(t + 1, 3 * g + e // 4)
                    if g < 2:
                        # k stats: sum k^2 (ACT), sum k*h (DVE ttr chain)
                        ksj = wp.tile([128, GRP], BF16, tag="junk", bufs=2, name="ksj")
                        nc.scalar.activation(
                            out=ksj[:], in_=mm_ps[:], func=AR.Square,
                            accum_out=sk_p[:, g : g + 1],
                        )
                        khj = wp.tile([128, GRP], BF16, tag="junk", bufs=2, name="khj")
                        nc.vector.scalar_tensor_tensor(
                            out=khj[:], in0=mm_ps[:], scalar=1.0,
                            in1=h_md[:, g * GRP : (g + 1) * GRP],
                            op0=ALU.mult, op1=ALU.mult,
                            accum_out=pk_c[g][:],
                        )
                    else:
                        vglo.append(mm_ps)

                # gate tail for this m-tile on [128,1]
                s1 = sp.tile([128, 1], F32, tag="s1")
                nc.scalar.activation(
                    out=s1[:], in_=sk_p[:, 0:1], func=AR.Identity,
                    bias=eps_sb[:, 0:1], scale=1.0 / D,
                )
                # add second k^2 part: s1 += sk_p[:,1]/D  (fold via stt)
                s1b = sp.tile([128, 1], F32, tag="s1b")
                nc.vector.scalar_tensor_tensor(
                    out=s1b[:], in0=sk_p[:, 1:2], scalar=1.0 / D, in1=s1[:],
                    op0=ALU.mult, op1=ALU.add,
                )
                s2 = sp.tile([128, 1], F32, tag="s2")
                nc.scalar.activation(
                    out=s2[:], in_=sh[:], func=AR.Identity,
                    bias=eps_sb[:, 0:1], scale=1.0 / D,
                )
                tt = sp.tile([128, 1], F32, tag="tt")
                nc.vector.tensor_mul(tt[:], s1b[:], s2[:])
                rr = sp.tile([128, 1], F32, tag="rr")
                nc.vector.reciprocal(rr[:], tt[:])
                rq = sp.tile([128, 1], F32, tag="rq")
                nc.scalar.activation(out=rq[:], in_=rr[:], func=AR.Sqrt)
                pks = sp.tile([128, 1], F32, tag="pks")
                nc.vector.tensor_add(pks[:], pk_c[0][:], pk_c[1][:])
                uu = sp.tile([128, 1], F32, tag="uu")
                nc.vector.scalar_tensor_tensor(
                    out=uu[:], in0=pks[:], scalar=float(1.0 / np.sqrt(D)),
                    in1=rq[:], op0=ALU.mult, op1=ALU.mult,
                )
                ab = sp.tile([128, 1], F32, tag="ab")
                nc.scalar.activation(out=ab[:], in_=uu[:], func=AR.Abs)
                mx = sp.tile([128, 1], F32, tag="mx")
                nc.vector.tensor_scalar_max(out=mx[:], in0=ab[:], scalar1=1e-6)
                r2 = sp.tile([128, 1], F32, tag="r2")
                nc.vector.reciprocal(r2[:], mx[:])
                q2 = sp.tile([128, 1], F32, tag="q2")
                nc.scalar.activation(out=q2[:], in_=r2[:], func=AR.Sqrt)
                st = sp.tile([128, 1], F32, tag="st")
                nc.vector.tensor_mul(st[:], uu[:], q2[:])
                nc.scalar.activation(
                    out=g_stats[:, t : t + 1], in_=st[:], func=AR.Sigmoid
                )

                # gated value -> v_md [m, d] bf16, transposed per group so
                # the PE transposes interleave with later matmul groups
                for gi, vp in enumerate(vglo):
                    nc.vector.tensor_scalar_mul(
                        out=v_md[:, gi * GRP : (gi + 1) * GRP], in0=vp[:],
                        scalar1=g_stats[:, t : t + 1],
                    )
                    for dt_ in range(gi * 8, (gi + 1) * 8):
                        pt = psp.tile([128, 128], BF16, tag="tpose", space="PSUM")
                        nc.tensor.transpose(
                            out=pt[:], in_=v_md[:, dt_ * 128 : (dt_ + 1) * 128],
                            identity=ident[:],
                        )
                        # last tile: only 24 token cols are live
                        cw_t = 128 if t < MT - 1 else 24
                        nc.scalar.copy(
                            out=v_sb[dt_][:, t * 128 : t * 128 + cw_t],
                            in_=pt[:, 0:cw_t],
                        )

                if t == 4:
                    conv_range(0)
                if t == 6:
                    conv_range(1)
                if t == 8:
                    conv_range(2)

            # final range: accumulate all 16 d-tiles into one tile and ship
            # a single 3D-AP DMA instead of 16 tiny ones (dispatch-bound)
            c0f, cwf = CONV_R[3]
            otb = wp.tile([128, DT, 48], BF16, tag="otb")
            for dt_ in range(DT):
                vs = v_sb[dt_]
                a1 = wp.tile([128, cwf], BF16, tag="a1", bufs=2)
                nc.vector.tensor_scalar(
                    out=a1[:], in0=vs[:, c0f : c0f + cwf],
                    scalar1=sc(dt_, SC_W0), scalar2=sc(dt_, SC_CB),
                    op0=ALU.mult, op1=ALU.add,
                )
                a2 = wp.tile([128, cwf], BF16, tag="a2", bufs=2)
                nc.vector.scalar_tensor_tensor(
                    out=a2[:], in0=vs[:, c0f + 8 : c0f + 8 + cwf],
                    scalar=sc(dt_, SC_W1), in1=a1[:],
                    op0=ALU.mult, op1=ALU.add,
                )
                a3 = wp.tile([128, cwf], BF16, tag="a3", bufs=2)
                nc.vector.scalar_tensor_tensor(
                    out=a3[:], in0=vs[:, c0f + 16 : c0f + 16 + cwf],
                    scalar=sc(dt_, SC_W2), in1=a2[:],
                    op0=ALU.mult, op1=ALU.add,
                )
                nc.vector.scalar_tensor_tensor(
                    out=otb[:, dt_, :], in0=vs[:, c0f + OFF : c0f + OFF + cwf],
                    scalar=sc(dt_, SC_W3P), in1=a3[:],
                    op0=ALU.mult, op1=ALU.add,
                )
            nc.sync.dma_start(
                outT.ap()[:, c0f : c0f + cwf].rearrange(
                    "(dt p) x -> p dt x", p=128
                ),
                otb[:, :, :],
            )

    _split_multi_waits(nc)
    return nc


_CACHE = {}


def _get_program():
    if "nc" not in _CACHE:
        _CACHE["nc"] = build_program()
    return _CACHE["nc"]


def host_prep(hidden_states, hash_input_ids, emb_tables, key_w, key_b,
              norm1_w, norm2_w, value_w, value_b, conv_w, conv_b):
    """Shard + lay out inputs for the 8 cores. Returns in_maps list."""
    bf = ml_dtypes.bfloat16
    w12 = norm1_w.astype(np.float64) * norm2_w.astype(np.float64)
    assert np.allclose(w12, 1.0, atol=1e-5), (
        "fast path assumes norm1_w*norm2_w == 1 (problem spec: fill=ones)"
    )
    assert not key_b.any() and not value_b.any(), (
        "fast path assumes zero key/value biases (problem spec: fill=zeros)"
    )

    tabs_np = np.ascontiguousarray(emb_tables.reshape(H * N, Dh)).astype(bf)
    wkv_np = np.empty((E, D2), bf)
    wkv_np[:, :D] = key_w.T.astype(bf)
    wkv_np[:, D:] = value_w.T.astype(bf)
    scal_d = np.empty((D, NSC), np.float32)
    scal_d[:, SC_W0] = conv_w[:, 0]
    scal_d[:, SC_W1] = conv_w[:, 1]
    scal_d[:, SC_W2] = conv_w[:, 2]
    scal_d[:, SC_W3P] = conv_w[:, 3] + 1.0
    scal_d[:, SC_CB] = conv_b
    scal_np = np.ascontiguousarray(
        scal_d.reshape(DT, 128, NSC).transpose(1, 0, 2).reshape(128, DT * NSC)
    )

    head_off = (np.arange(H, dtype=np.int64) * N)[None, :]
    OOB = np.int32(H * N)

    in_maps = []
    for c in range(NCORES):
        l0 = c * LC
        lo = l0 - HALO
        lo_clip = max(lo, 0)
        nvalid = (l0 + LC) - lo_clip
        r0 = (lo_clip - lo) * B
        ids_c = np.full((MP, H), OOB, np.int32)
        seg = hash_input_ids[lo_clip : l0 + LC].reshape(nvalid * B, H)
        ids_c[r0 : r0 + nvalid * B] = (seg.astype(np.int64) + head_off).astype(
            np.int32
        )
        hid_c = np.zeros((MP, D), bf)
        hseg = hidden_states[lo_clip : l0 + LC].reshape(nvalid * B, D)
        hid_c[r0 : r0 + nvalid * B] = hseg.astype(bf)
        ids_r = np.ascontiguousarray(
            ids_c.reshape(MT, 128, H).transpose(1, 0, 2).reshape(128, MT * H)
        )
        in_maps.append(
            {
                "tabs": tabs_np,
                "ids": ids_r,
                "hid": hid_c,
                "wkv": wkv_np,
                "scal": scal_np,
            }
        )
    return in_maps


def unshard_output(results):
    """results: list of per-core dicts with 'outT' [D, MOUT] -> [L, B, D]."""
    out = np.empty((L, B, D), np.float32)
    for c in range(NCORES):
        o = results[c]["outT"].astype(np.float32)
        out[c * LC : (c + 1) * LC] = o.reshape(D, LC, B).transpose(1, 2, 0)
    return out


def kernel(hidden_states, hash_input_ids, emb_tables, key_w, key_b,
           norm1_w, norm2_w, value_w, value_b, conv_w, conv_b):
    args = [hidden_states, hash_input_ids, emb_tables, key_w, key_b,
            norm1_w, norm2_w, value_w, value_b, conv_w, conv_b]
    args = [np.asarray(a) for a in args]
    in_maps = host_prep(*args)
    nc = _get_program()
    res = run_bass_kernel_spmd(nc, in_maps, list(range(NCORES)))
    return unshard_output(res.results)

